# revision 4
# baseline (speedup 1.0000x reference)
"""AdjacencyProjector kernel for 8 Trainium2 NeuronCores.

score[b, i, j] = E[b, i] . W[0, :D]  +  E[b, j] . W[0, D:]

B=4, N=4096, D=128. Output (4, 4096, 4096) f32 = 256MB -> memory (write)
bound. Sharding: 8 cores x (batch, row-half): core k computes rows
[h*2048, (h+1)*2048) of batch b where b = k//2, h = k%2.

Each core receives the full batch E (2MB) ROLLED so its own 2048 rows
come first; the kernel computes with static offsets and emits output
columns in the same rolled order; the host un-rolls the columns when
gathering. The kernel streams the 32MB output shard in column halves so
output DMA starts as soon as the first half of bvec is ready.
"""

import sys

sys.path.insert(0, "/opt/trn_rl_repo")

import numpy as np

B, N, D = 4, 4096, 128
P = 128
ROWS_PER_CORE = N // 2          # 2048
NT = N // P                     # 32 column chunks per batch
NR = ROWS_PER_CORE // P         # 16 row blocks per core
HALF = N // 2                   # 2048 columns per half
NTH = NT // 2                   # 16 column chunks per half
N_CORES = 8

_CACHE = {}


def _build_nc():
    import concourse.bacc as bacc
    import concourse.bass as bass
    import concourse.mybir as mybir
    from concourse.tile import TileContext
    from concourse.masks import make_identity

    f32 = mybir.dt.float32
    nc = bacc.Bacc("TRN2", num_devices=N_CORES)

    eb_d = nc.declare_dram_parameter("Eb", [N, D], f32, isOutput=False)
    w_d = nc.declare_dram_parameter("W", [1, 2 * D], f32, isOutput=False)
    out_d = nc.declare_dram_parameter("out", [ROWS_PER_CORE, N], f32, isOutput=True)

    def bcast_free(ap, n, at=1):
        # insert a stride-0 dim of size n at free position `at`
        return bass.AP(
            tensor=ap.tensor,
            offset=ap.offset,
            ap=ap.ap[:at] + [[0, n]] + ap.ap[at:],
        )

    with TileContext(nc) as tc:
        with (
            tc.tile_pool(name="consts", bufs=1) as consts,
            tc.tile_pool(name="work", bufs=1) as work,
            tc.tile_pool(name="psum", bufs=2, space="PSUM") as psum,
            tc.tile_pool(name="outp", bufs=6) as outp,
        ):
            ident = consts.tile([P, P], f32)
            make_identity(nc, ident)
            ones = consts.tile([1, P], f32)
            nc.vector.memset(ones, 1.0)

            wi_rep = consts.tile([P, D], f32)
            nc.gpsimd.dma_start(
                out=wi_rep, in_=w_d.ap()[0:1, 0:D].partition_broadcast(P)
            )
            wj_rep = consts.tile([P, D], f32)
            nc.gpsimd.dma_start(
                out=wj_rep, in_=w_d.ap()[0:1, D : 2 * D].partition_broadcast(P)
            )

            eb_tiled = eb_d.ap().rearrange("(t p) d -> p t d", p=P)

            # per column half s: load E rows, bvec half, brep half
            ebh = []
            brep = []
            for s in range(2):
                e = work.tile([P, NTH, D], f32, tag=f"ebh{s}")
                nc.sync.dma_start(out=e, in_=eb_tiled[:, s * NTH : (s + 1) * NTH, :])
                ebh.append(e)

            # ---- row term from first half (the core's own rows):
            # avec[i] = Eb[i] . wi for i in [0, 2048)
            prod2 = work.tile([P, NTH, D], f32)
            nc.vector.tensor_mul(
                out=prod2, in0=ebh[0], in1=bcast_free(wi_rep[:], NTH)
            )
            acols = work.tile([P, NR], f32)
            nc.vector.tensor_reduce(
                out=acols,
                in_=prod2,
                axis=mybir.AxisListType.X,
                op=mybir.AluOpType.add,
            )

            for s in range(2):
                prod = work.tile([P, NTH, D], f32, tag=f"prod{s}")
                nc.vector.tensor_mul(
                    out=prod, in0=ebh[s], in1=bcast_free(wj_rep[:], NTH)
                )
                bcols = work.tile([P, NTH], f32, tag=f"bcols{s}")
                nc.vector.tensor_reduce(
                    out=bcols,
                    in_=prod,
                    axis=mybir.AxisListType.X,
                    op=mybir.AluOpType.add,
                )
                btp = psum.tile([NTH, P], f32, tag=f"btp{s}")
                nc.tensor.transpose(btp[:], bcols[:], ident[:])
                bt = work.tile([NTH, P], f32, tag=f"bt{s}")
                nc.scalar.copy(out=bt, in_=btp)
                brow = work.tile([1, HALF], f32, tag=f"brow{s}")
                nc.sync.dma_start(out=brow[:], in_=bt[:])

                br = work.tile([P, HALF], f32, tag=f"brep{s}")
                for g in range(4):
                    pb = psum.tile([P, 512], f32, tag="pb")
                    nc.tensor.matmul(
                        pb[:],
                        ones[:],
                        brow[0:1, g * 512 : (g + 1) * 512],
                        start=True,
                        stop=True,
                    )
                    nc.vector.tensor_copy(
                        out=br[:, g * 512 : (g + 1) * 512], in_=pb
                    )
                brep.append(br)

            # ---- output tiles, column half s of row block r:
            # out[r*128+p, s*2048+j] = brep[s][p, j] + avec[r*128+p]
            idx = 0
            for s in range(2):
                for r in range(NR):
                    ot = outp.tile([P, HALF], f32, tag="ot")
                    if idx % 3 == 2:
                        nc.scalar.add(ot[:], brep[s][:], acols[:, r : r + 1])
                    else:
                        nc.vector.tensor_scalar_add(
                            ot[:], brep[s][:], acols[:, r : r + 1]
                        )
                    dma = nc.sync if idx % 2 == 0 else nc.gpsimd
                    dma.dma_start(
                        out=out_d.ap()[
                            r * P : (r + 1) * P, s * HALF : (s + 1) * HALF
                        ],
                        in_=ot,
                    )
                    idx += 1

    nc.compile()
    return nc


def _get_nc():
    if "nc" not in _CACHE:
        _CACHE["nc"] = _build_nc()
    return _CACHE["nc"]


def _run(E, W, trace=False, tmpdir=None):
    from concourse.bass_utils import run_bass_kernel_spmd

    E = np.asarray(E, dtype=np.float32)
    W = np.asarray(W, dtype=np.float32)
    nc = _get_nc()

    in_maps = []
    for k in range(N_CORES):
        b, h = k // 2, k % 2
        if h == 0:
            eb = E[b]
        else:
            eb = np.concatenate([E[b, HALF:], E[b, :HALF]], axis=0)
        in_maps.append({"Eb": np.ascontiguousarray(eb), "W": W})
    res = run_bass_kernel_spmd(
        nc, in_maps, core_ids=list(range(N_CORES)), trace=trace, tmpdir=tmpdir
    )
    out = np.empty((B, N, N), dtype=np.float32)
    for k in range(N_CORES):
        b, h = k // 2, k % 2
        r = res.results[k]["out"]
        rows = slice(h * ROWS_PER_CORE, (h + 1) * ROWS_PER_CORE)
        if h == 0:
            out[b, rows, :] = r
        else:
            out[b, rows, :HALF] = r[:, HALF:]
            out[b, rows, HALF:] = r[:, :HALF]
    return out, res


def kernel(E, W):
    out, _ = _run(E, W)
    return out
